# revision 7
# baseline (speedup 1.0000x reference)
"""Distributed Trainium2 kernel for nn_DecoderAttentionRotary.

Strategy (8 NeuronCores, tensor-parallel over heads):
  - host: transpose x -> xT [D, B*L]; per-core Wqkv column slice reordered to
    [q0,k0,q1,k1,v0,v1]; cos/sin transposed + batch-tiled; causal masks.
  - device, per core (2 heads):
      phase 1: qkT = (Wqk^T @ xT) + b  (f32r matmuls, K-tiled),
               v = x @ Wv directly in [l, hd] layout (xT-chunks stationary)
      RoPE on first 32 rows of each q/k block (DVE + small SBUF shuffle DMAs)
      phase 2: causal attention per (batch, head) in scores^T layout:
               scoresT[k,q] blocks -> exp (ACT) -> mask (DVE) ->
               out^T accum + ones-matmul rowsums (PE) -> normalize
      phase 3: AllToAll reshard (head-shards -> row-shards), then
               y rows = outT_rows^T @ Wd + bd  (per-core 512-row slice)
  - host: concatenate row slices.
"""
import sys

for _p in ("/opt/pypackages", "/opt/trn_rl_repo"):
    if _p not in sys.path:
        sys.path.insert(0, _p)

import numpy as np

B, L, D, H = 2, 2048, 2048, 16
HD, R = 128, 32
SCALE = float(HD) ** -0.5
W = 8
HPC = H // W              # heads per core
M = B * L                 # flattened rows
CORES = list(range(W))

_NC = None


def _build_nc():
    import concourse.mybir as mybir
    import concourse.tile as tile
    from concourse import bacc

    f32 = mybir.dt.float32
    f32r = mybir.dt.float32r
    AFT = mybir.ActivationFunctionType
    OP = mybir.AluOpType

    nc = bacc.Bacc(None, target_bir_lowering=False, num_devices=W)
    xT = nc.declare_dram_parameter("xT", [D, M], f32r, isOutput=False)
    wqkv = nc.declare_dram_parameter("wqkv", [D, 6 * HD], f32r, isOutput=False)
    bqk = nc.declare_dram_parameter("bqk", [4 * HD, 1], f32, isOutput=False)
    bv = nc.declare_dram_parameter("bv", [1, 2 * HD], f32, isOutput=False)
    cosT = nc.declare_dram_parameter("cosT", [R, M], f32, isOutput=False)
    sinT = nc.declare_dram_parameter("sinT", [R, M], f32, isOutput=False)
    masks = nc.declare_dram_parameter("masks", [4, 128, 512], f32r, isOutput=False)
    wd = nc.declare_dram_parameter("wd", [D, D], f32r, isOutput=False)
    bdb = nc.declare_dram_parameter("bdb", [128, D], f32, isOutput=False)
    onesc = nc.declare_dram_parameter("onesc", [128, 1], f32r, isOutput=False)
    y = nc.declare_dram_parameter("y", [M // W, D], f32, isOutput=True)

    xT_r = xT.ap().rearrange("(t p) n -> p t n", p=128)   # [128, 16, M]

    with tile.TileContext(nc) as tc:
        with (
            tc.tile_pool(name="const", bufs=1) as cpool,
            tc.tile_pool(name="dram", bufs=1, space="DRAM") as dpool,
        ):
            a2a_in = dpool.tile([W, HPC * HD, 512], f32r)
            a2a_out = dpool.tile([W, HPC * HD, 512], f32r)

            w_sb = cpool.tile([128, 16, 6 * HD], f32r)
            nc.sync.dma_start(
                out=w_sb[:], in_=wqkv.ap().rearrange("(t p) m -> p t m", p=128)
            )
            cos_sb = cpool.tile([R, M], f32)
            nc.sync.dma_start(out=cos_sb[:], in_=cosT.ap())
            sin_sb = cpool.tile([R, M], f32)
            nc.sync.dma_start(out=sin_sb[:], in_=sinT.ap())
            mask_sb = cpool.tile([128, 4, 512], f32r)
            nc.sync.dma_start(
                out=mask_sb[:], in_=masks.ap().rearrange("j p n -> p j n")
            )
            bqk_sb = cpool.tile([128, 4], f32)
            nc.sync.dma_start(
                out=bqk_sb[:], in_=bqk.ap().rearrange("(t p) o -> p (t o)", p=128)
            )
            bv_sb = cpool.tile([1, 2 * HD], f32)
            nc.sync.dma_start(out=bv_sb[:], in_=bv.ap())
            ones_r = cpool.tile([1, 128], f32)
            nc.vector.memset(ones_r[:], 1.0)
            ones_c = cpool.tile([128, 1], f32r)
            nc.sync.dma_start(out=ones_c[:], in_=onesc.ap())

            for b in range(B):
                with (
                    tc.tile_pool(name=f"qk{b}", bufs=1) as qkpool,
                    tc.tile_pool(name=f"v{b}", bufs=1) as vpool,
                ):
                    qk_sb = qkpool.tile([128, 4, L], f32r)     # q0,k0,q1,k1 ^T
                    v_sb = vpool.tile([128, 16, 2 * HD], f32r)  # [l-part, l-tile, v0|v1]

                    # ---- phase 1: projections ----
                    with (
                        tc.tile_pool(name="xt", bufs=3) as xtpool,
                        tc.tile_pool(name="p1a", bufs=4, space="PSUM") as qkps_pool,
                        tc.tile_pool(name="p1b", bufs=4, space="PSUM") as vps_pool,
                    ):
                        for nch in range(L // 512):
                            n0 = b * L + nch * 512
                            xt_tiles = []
                            for half in range(2):
                                xt = xtpool.tile([128, 8, 512], f32r, tag="xt")
                                nc.sync.dma_start(
                                    out=xt[:],
                                    in_=xT_r[:, half * 8:(half + 1) * 8, n0:n0 + 512],
                                )
                                xt_tiles.append(xt)
                            for m in range(4):
                                ps = qkps_pool.tile([128, 512], f32, tag="qkps")
                                for kt in range(16):
                                    xt = xt_tiles[kt // 8]
                                    nc.tensor.matmul(
                                        ps[:],
                                        lhsT=w_sb[:, kt, m * 128:(m + 1) * 128],
                                        rhs=xt[:, kt % 8, :],
                                        start=(kt == 0),
                                        stop=(kt == 15),
                                    )
                                nc.vector.tensor_scalar_add(
                                    qk_sb[:, m, nch * 512:(nch + 1) * 512],
                                    ps[:],
                                    bqk_sb[:, m:m + 1],
                                )
                            for rr in range(4):
                                vps = vps_pool.tile([128, 2 * HD], f32, tag="vps")
                                for kt in range(16):
                                    xt = xt_tiles[kt // 8]
                                    nc.tensor.matmul(
                                        vps[:],
                                        lhsT=xt[:, kt % 8, rr * 128:(rr + 1) * 128],
                                        rhs=w_sb[:, kt, 4 * HD:6 * HD],
                                        start=(kt == 0),
                                        stop=False,
                                    )
                                nc.tensor.matmul(
                                    vps[:], lhsT=ones_r[:], rhs=bv_sb[:],
                                    start=False, stop=True,
                                )
                                nc.scalar.activation(
                                    v_sb[:, nch * 4 + rr, :], vps[:], AFT.Copy
                                )

                    # ---- RoPE (in place on qk_sb rows 0:R) ----
                    with tc.tile_pool(name="rope", bufs=2) as rpool:
                        cs = cos_sb[:, b * L:(b + 1) * L]
                        sn = sin_sb[:, b * L:(b + 1) * L]
                        for m in range(4):
                            ta = rpool.tile([R, L], f32, tag="ta")
                            rot = rpool.tile([R, L], f32, tag="rot")
                            # rot = rotate_half(q_rot): rows 0:16 <- q[16:32], 16:32 <- q[0:16]
                            nc.sync.dma_start(
                                out=rot[0:16, :], in_=qk_sb[16:32, m, :].bitcast(f32)
                            )
                            nc.sync.dma_start(
                                out=rot[16:32, :], in_=qk_sb[0:16, m, :].bitcast(f32)
                            )
                            nc.vector.tensor_tensor(
                                ta[:], qk_sb[0:R, m, :], cs, op=OP.mult
                            )
                            nc.vector.tensor_tensor(rot[:], rot[:], sn, op=OP.mult)
                            nc.vector.tensor_tensor(
                                qk_sb[0:R, m, :], ta[:], rot[:], op=OP.add
                            )

                    # ---- phase 2: attention ----
                    with (
                        tc.tile_pool(name="att", bufs=3) as apool,
                        tc.tile_pool(name="attps", bufs=3, space="PSUM") as apsum,
                    ):
                        for h in range(HPC):
                            for qc in range(L // 512):
                                nk = 4 * qc + 4
                                outp = apsum.tile([128, 512], f32, tag="outp", bufs=2)
                                sump = apsum.tile([1, 512], f32, tag="sump", bufs=2)
                                for ki in range(nk):
                                    sp = apsum.tile([128, 512], f32, tag="sc", bufs=3)
                                    nc.tensor.matmul(
                                        sp[:],
                                        lhsT=qk_sb[:, 2 * h + 1, ki * 128:(ki + 1) * 128],
                                        rhs=qk_sb[:, 2 * h, qc * 512:(qc + 1) * 512],
                                        start=True, stop=True,
                                    )
                                    et = apool.tile([128, 512], f32r, tag="et")
                                    nc.scalar.activation(et[:], sp[:], AFT.Exp, scale=SCALE)
                                    if ki >= qc * 4:
                                        nc.vector.tensor_tensor(
                                            et[:], et[:], mask_sb[:, ki - qc * 4, :],
                                            op=OP.mult,
                                        )
                                    nc.tensor.matmul(
                                        outp[:],
                                        lhsT=v_sb[:, ki, h * 128:(h + 1) * 128],
                                        rhs=et[:],
                                        start=(ki == 0), stop=(ki == nk - 1),
                                    )
                                    nc.tensor.matmul(
                                        sump[:], lhsT=ones_c[:], rhs=et[:],
                                        start=(ki == 0), stop=(ki == nk - 1),
                                    )
                                rec = apool.tile([1, 512], f32, tag="rec")
                                nc.vector.reciprocal(rec[:], sump[:])
                                bc = apsum.tile([128, 512], f32, tag="bc", bufs=1)
                                nc.tensor.matmul(bc[:], lhsT=ones_r[:], rhs=rec[:])
                                bcs = apool.tile([128, 512], f32, tag="bcs")
                                nc.scalar.activation(bcs[:], bc[:], AFT.Copy)
                                ot = apool.tile([128, 512], f32r, tag="ot")
                                nc.vector.tensor_tensor(
                                    ot[:], outp[:], bcs[:], op=OP.mult
                                )
                                slot = b * 4 + qc
                                nc.sync.dma_start(
                                    out=a2a_in[slot, h * 128:(h + 1) * 128, :],
                                    in_=ot[:],
                                )

            # ---- phase 3: A2A reshard + output projection ----
            nc.gpsimd.collective_compute(
                "AllToAll",
                mybir.AluOpType.bypass,
                replica_groups=[CORES],
                ins=[a2a_in[:]],
                outs=[a2a_out[:]],
            )
            with (
                tc.tile_pool(name="p3c", bufs=1) as p3c,
                tc.tile_pool(name="p3", bufs=3) as p3pool,
                tc.tile_pool(name="p3ps", bufs=8, space="PSUM") as yps_pool,
            ):
                o_sb = p3c.tile([128, 16, 512], f32r)
                nc.sync.dma_start(
                    out=o_sb[:],
                    in_=a2a_out[:].rearrange("j (u p) n -> p (j u) n", p=128),
                )
                bd_sb = p3c.tile([128, D], f32)
                nc.sync.dma_start(out=bd_sb[:], in_=bdb.ap())
                for n4 in range(4):
                    yps = [
                        yps_pool.tile([128, 512], f32, tag="yps", name=f"yps{n4}_{m}")
                        for m in range(4)
                    ]
                    for kt in range(16):
                        wt = p3pool.tile([128, 512], f32r, tag="wt")
                        nc.sync.dma_start(
                            out=wt[:],
                            in_=wd[kt * 128:(kt + 1) * 128, n4 * 512:(n4 + 1) * 512],
                        )
                        for m in range(4):
                            nc.tensor.matmul(
                                yps[m][:],
                                lhsT=o_sb[:, kt, m * 128:(m + 1) * 128],
                                rhs=wt[:],
                                start=(kt == 0), stop=(kt == 15),
                            )
                    for m in range(4):
                        yt = p3pool.tile([128, 512], f32, tag="yt")
                        nc.vector.tensor_tensor(
                            yt[:], yps[m][:], bd_sb[:, n4 * 512:(n4 + 1) * 512],
                            op=OP.add,
                        )
                        nc.sync.dma_start(
                            out=y[m * 128:(m + 1) * 128, n4 * 512:(n4 + 1) * 512],
                            in_=yt[:],
                        )
    nc.finalize()
    return nc


def _host_prep(x_BLD, cos, sin, Wqkv, bqkv, Wd, bd):
    x = np.ascontiguousarray(np.asarray(x_BLD, np.float32).reshape(M, D))
    xT = np.ascontiguousarray(x.T)
    c2 = np.asarray(cos, np.float32).reshape(L, R).T
    s2 = np.asarray(sin, np.float32).reshape(L, R).T
    cosT = np.ascontiguousarray(np.concatenate([c2] * B, axis=1))
    sinT = np.concatenate([s2] * B, axis=1)
    sinT_pm = np.ascontiguousarray(
        np.concatenate([-sinT[:16], sinT[16:]], axis=0)
    )
    kk = np.arange(128, dtype=np.int64)[:, None]
    qq = np.arange(512, dtype=np.int64)[None, :]
    masks = np.stack(
        [(qq >= j * 128 + kk).astype(np.float32) for j in range(4)]
    )
    bdb = np.ascontiguousarray(
        np.broadcast_to(np.asarray(bd, np.float32), (128, D))
    )
    Wqkv = np.asarray(Wqkv, np.float32)
    bqkv = np.asarray(bqkv, np.float32)
    in_maps = []
    for c in range(W):
        base = c * HPC * 3 * HD
        qk_idx = np.concatenate(
            [np.arange(base + h * 3 * HD, base + h * 3 * HD + 2 * HD)
             for h in range(HPC)]
        )
        v_idx = np.concatenate(
            [np.arange(base + h * 3 * HD + 2 * HD, base + (h + 1) * 3 * HD)
             for h in range(HPC)]
        )
        idx = np.concatenate([qk_idx, v_idx])
        in_maps.append({
            "xT": xT,
            "wqkv": np.ascontiguousarray(Wqkv[:, idx]),
            "bqk": np.ascontiguousarray(bqkv[qk_idx].reshape(4 * HD, 1)),
            "bv": np.ascontiguousarray(bqkv[v_idx].reshape(1, 2 * HD)),
            "cosT": cosT,
            "sinT": sinT_pm,
            "masks": masks,
            "wd": np.asarray(Wd, np.float32),
            "bdb": bdb,
            "onesc": np.ones((128, 1), np.float32),
        })
    return in_maps


def _get_nc():
    global _NC
    if _NC is None:
        _NC = _build_nc()
    return _NC


def _run(inputs, trace=False, tmpdir=None):
    from concourse.bass_utils import run_bass_kernel_spmd

    in_maps = _host_prep(**inputs)
    nc = _get_nc()
    res = run_bass_kernel_spmd(nc, in_maps, CORES, trace=trace, tmpdir=tmpdir)
    yb = np.concatenate([res.results[c]["y"] for c in CORES], axis=0)
    return yb.reshape(B, L, D), res


def kernel(**inputs) -> np.ndarray:
    out, _ = _run(inputs)
    return out
